# revision 32
# baseline (speedup 1.0000x reference)
"""GAT layer on 8 Trainium2 NeuronCores (Bass/Tile).

Strategy (edge partition by dst, 1D graph parallelism), v2:
- Each core owns 6250 dst nodes; nodes re-ordered by in-degree so each
  128-node PSUM window has near-uniform degree; edge slot (p, c) holds the
  c-th incoming edge of window-node p, so the segment scatter-sum is a
  matmul-accumulate with an identity stationary operand and er[dst]
  broadcasts per-partition.
- Projection h = featT.T @ [W | Wl | Wr] is node-parallel in bf16 with the
  feat tensor pre-transposed host-side (no on-chip transposes).  h is
  stored feature-major (lane = f*8 + head) so the per-edge softmax scale
  broadcast has a packed inner dim (DVE 2x mode).
- Per-node table row (768 B pitch): [el f32 32B | h bf16 512B | pad].
  The table is AllGathered in 7 chunks overlapped with the projection.
- Table halves for int16 gather indices split by rank parity (even/odd
  rows via stride-1536B views); within each window the 128 nodes are
  split 64/64 into parity classes balancing every dst node's in-edges
  across halves (keeps per-window grid columns ~deg/2 per half).
- leaky-relu and exp run on the scalar engine; softmax is unnormalized
  (logits are O(1)) and normalization happens per node after aggregation.
"""
import math
import numpy as np
import sys

sys.path.insert(0, "/opt/trn_rl_repo")

from concourse import bass, mybir, bacc, tile
from concourse.bass_utils import run_bass_kernel_spmd

N_NODES = 50000
N_EDGES = 800000
IN_FEATS = 256
NUM_HEADS = 8
OUT_FEATS = 32
HF = NUM_HEADS * OUT_FEATS          # 256
NEG_SLOPE = 0.2
N_CORES = 8
NPAD = 6400                         # 50 * 128 ranks per core
NWIN = NPAD // 128                  # 50
NCHUNK = 2                          # AllGather chunks == table halves
WPC = NWIN // NCHUNK                # 25 windows per chunk
RPC = WPC * 128                     # 3200 rows per (core, chunk)
ROWT = 384                          # table row u16 lanes (768 B):
                                    #   lane 0-15:    el f32 (8 heads)
                                    #   lane 16-271:  h bf16 (fh layout)
                                    #   lane 272-383: pad
TROWS = N_CORES * NPAD              # 51200
HALF = TROWS // 2                   # 25600 rows per half (chunks 0-4 / 5-9)
HRANK = NPAD // 2                   # 3200 ranks per half per core
HREAL = N_NODES // (2 * N_CORES)    # 3125 real nodes per (core, half)
PAD_P = HREAL - (HRANK // 128 - 1) * 128   # 53: pad partition threshold
PADW = (NWIN // 2 - 1, NWIN - 1)    # windows 24 and 49 hold the pad ranks
PAD_EL = -1e30                      # pad rows' el -> exp() == 0
MAXC = 8                            # chunk columns per dma_gather
NSQ = 4                             # SWDGE queues
ROWC = 272                          # used lanes per row (el + h)
F32 = mybir.dt.float32
BF16 = mybir.dt.bfloat16
U16 = mybir.dt.uint16
I16 = mybir.dt.int16


def _row_of(core, rank):
    """Table row index for (core, rank-within-core) under chunked AG."""
    k = rank // RPC
    return k * (N_CORES * RPC) + core * RPC + (rank % RPC)


def _assign(src, dst):
    """Assign each node a (core, half).  Nodes are processed in degree-desc
    groups of 16; each group fills the 16 (core, half) slots, the greedy
    choosing each node's half to balance every dst node's in-edge count
    across table halves, followed by swap-refinement sweeps.  Every
    (core, half) holds exactly HREAL real nodes (stratified by degree)."""
    deg = np.bincount(dst, minlength=N_NODES)
    order_by_src = np.argsort(src, kind="stable")
    ssd = dst[order_by_src]
    starts = np.searchsorted(src[order_by_src], np.arange(N_NODES + 1))

    gorder = np.argsort(-deg, kind="stable")
    imb = np.zeros(N_NODES, np.int32)       # degA - degB per dst node
    core_of = np.empty(N_NODES, np.int64)
    half_of = np.empty(N_NODES, np.int64)
    for g in range(HREAL):
        grp = gorder[g * 16:(g + 1) * 16]
        capA = capB = 8
        for v in grp:
            nb = ssd[starts[v]:starts[v + 1]]
            if capA == 0:
                toA = False
            elif capB == 0:
                toA = True
            elif len(nb) == 0:
                toA = capA >= capB
            else:
                cur = imb[nb]
                toA = np.abs(cur + 1).sum() <= np.abs(cur - 1).sum()
            if toA:
                capA -= 1
                core_of[v] = 7 - capA
                half_of[v] = 0
                if len(nb):
                    imb[nb] += 1
            else:
                capB -= 1
                core_of[v] = 7 - capB
                half_of[v] = 1
                if len(nb):
                    imb[nb] -= 1
    # swap-refinement: within each group, swap the best (A, B) pair's
    # (core, half) labels while it reduces total imbalance
    for sweep in range(6):
        nswap = 0
        for g in range(HREAL):
            grp = gorder[g * 16:(g + 1) * 16]
            ha = half_of[grp]
            A = grp[ha == 0]
            B = grp[ha == 1]
            gA = np.empty(len(A))
            gB = np.empty(len(B))
            for i, u in enumerate(A):
                cur = imb[ssd[starts[u]:starts[u + 1]]]
                gA[i] = (np.abs(cur - 2) - np.abs(cur)).sum()
            for i, v in enumerate(B):
                cur = imb[ssd[starts[v]:starts[v + 1]]]
                gB[i] = (np.abs(cur + 2) - np.abs(cur)).sum()
            iu = int(gA.argmin())
            iv = int(gB.argmin())
            if gA[iu] + gB[iv] < 0:
                u, v = A[iu], B[iv]
                imb[ssd[starts[u]:starts[u + 1]]] -= 2
                imb[ssd[starts[v]:starts[v + 1]]] += 2
                half_of[u], half_of[v] = 1, 0
                core_of[u], core_of[v] = core_of[v], core_of[u]
                nswap += 1
        if nswap == 0:
            break
    return core_of, half_of


def _grid_for(p_arr, rel_arr):
    """Edges (partition p, rel table idx) -> grid [128, ncx] int32,
    pad slots = -1.  Each partition's edges sorted ascending by rel."""
    if len(p_arr) == 0:
        return np.full((128, 1), -1, np.int32)
    o = np.lexsort((rel_arr, p_arr))
    p_arr, rel_arr = p_arr[o], rel_arr[o]
    counts = np.bincount(p_arr, minlength=128)
    ncx = int(counts.max())
    grid = np.full((128, ncx), -1, np.int32)
    starts = np.zeros(128, np.int64)
    starts[1:] = np.cumsum(counts)[:-1]
    col = np.arange(len(p_arr)) - starts[p_arr]
    grid[p_arr, col] = rel_arr
    return grid


def _pack_gather(grid_cols):
    """grid_cols [128, cols] -> wrapped idx image [128, n/16] int16."""
    cols = grid_cols.shape[1]
    n = 128 * cols
    flat = np.empty(n, np.int32)
    P = np.arange(128)[:, None]
    CI = np.arange(cols)[None, :]
    pos = (P % 16) * (n // 16) + (P // 16) + 8 * CI
    flat[pos.ravel()] = grid_cols.ravel()
    return np.tile(flat.reshape(16, n // 16), (8, 1)).astype(np.int16)


def _prep(src, dst):
    """Host-side index preprocessing.  Returns per-core node perms + the
    shared gather schedule + per-core idx images."""
    core_of, half_of = _assign(src, dst)

    # rank within each (core, half) by (max(degA,degB) desc, min desc): the
    # grid column count per (window, half) is the window max of each
    # half-degree, so grouping by the worst-half degree minimizes both
    srcA = half_of[src] == 0
    degA = np.bincount(dst[srcA], minlength=N_NODES)
    degB = np.bincount(dst[~srcA], minlength=N_NODES)
    kmax = np.maximum(degA, degB)
    kmin = np.minimum(degA, degB)
    rank_of = np.empty(N_NODES, np.int64)
    for c in range(N_CORES):
        for h in range(2):
            ids = np.nonzero((core_of == c) & (half_of == h))[0]
            order = np.lexsort((-kmin[ids], -kmax[ids]))
            rank_of[ids[order]] = h * HRANK + np.arange(len(ids))

    perms = []                              # rank -> global node id
    for c in range(N_CORES):
        perm = np.full(NPAD, -1, np.int64)
        ids = np.nonzero(core_of == c)[0]
        perm[rank_of[ids]] = ids
        perms.append(perm)

    row_all = (rank_of // RPC) * (N_CORES * RPC) \
        + core_of * RPC + (rank_of % RPC)

    sg_half = (rank_of[src] >= HRANK).astype(np.int64)
    sg_rel = row_all[src] - sg_half * HALF  # rel row within half
    dcore = core_of[dst]
    dr_all = rank_of[dst]                   # local rank of dst

    # per (core, window, half) grids
    grids = [[[None, None] for _ in range(NWIN)] for _ in range(N_CORES)]
    ncx = np.zeros((N_CORES, NWIN, 2), np.int64)
    for c in range(N_CORES):
        sel = np.nonzero(dcore == c)[0]
        rel = sg_rel[sel]
        half = sg_half[sel]
        dr = dr_all[sel]
        w_arr = dr // 128
        p_arr = dr % 128
        key = w_arr * 2 + half
        order = np.argsort(key, kind="stable")
        ksort = key[order]
        bounds = np.searchsorted(ksort, np.arange(NWIN * 2 + 1))
        for w in range(NWIN):
            for h in range(2):
                lo, hi = bounds[w * 2 + h], bounds[w * 2 + h + 1]
                idxs = order[lo:hi]
                g = _grid_for(p_arr[idxs].astype(np.int64),
                              rel[idxs].astype(np.int64))
                grids[c][w][h] = g
                ncx[c, w, h] = g.shape[1]

    # shared schedule: per (window, half) chunk count = max over cores
    ncE = np.maximum(ncx[:, :, 0].max(axis=0), 1)
    ncO = np.maximum(ncx[:, :, 1].max(axis=0), 1)
    sched = []                              # (w, half, cols, off16)
    off16 = 0
    for w in range(NWIN):
        for h, nc_w in ((0, int(ncE[w])), (1, int(ncO[w]))):
            for j0 in range(0, nc_w, MAXC):
                cols = min(MAXC, nc_w - j0)
                sched.append((w, h, cols, off16, j0))
                off16 += cols * 8           # n/16 = 128*cols/16
    toti16 = off16
    queues = [i % NSQ for i in range(len(sched))]

    # pad slots rotate over all pad rows of the matching half (ranks
    # HREAL..HRANK / HRANK+HREAL..NPAD) to dodge same-bank serialization.
    padpools = []
    for h in range(2):
        pool = []
        for c in range(N_CORES):
            for r in range(h * HRANK + HREAL, (h + 1) * HRANK):
                pool.append(_row_of(c, r) - h * HALF)
        padpools.append(np.array(pool, np.int32))

    idx_imgs = []
    for c in range(N_CORES):
        img = np.empty((128, toti16), np.int16)
        phase = 0
        for (w, h, cols, off, j0) in sched:
            g = grids[c][w][h]
            gc = np.full((128, cols), -1, np.int32)
            avail = max(0, min(cols, g.shape[1] - j0))
            if avail > 0:
                gc[:, :avail] = g[:, j0:j0 + avail]
            mask = gc < 0
            npads = int(mask.sum())
            if npads:
                pool = padpools[h]
                gc[mask] = pool[(np.arange(npads) + phase) % len(pool)]
                phase += npads
            img[:, off:off + cols * 8] = _pack_gather(gc)
        idx_imgs.append(img)
    return perms, sched, toti16, idx_imgs, ncE, ncO, queues


def _build(sched, toti16, ncE, ncO, queues):
    nc = bacc.Bacc("TRN2", target_bir_lowering=False, debug=False,
                   num_devices=N_CORES, num_swdge_queues=NSQ)
    featT_in = nc.dram_tensor("featT", [IN_FEATS, NPAD], BF16,
                              kind="ExternalInput")
    w_in = nc.dram_tensor("w", [IN_FEATS, HF], BF16, kind="ExternalInput")
    alb_in = nc.dram_tensor("alb", [128, HF], F32, kind="ExternalInput")
    arb_in = nc.dram_tensor("arb", [128, HF], F32, kind="ExternalInput")
    bias_in = nc.dram_tensor("biasb", [128, HF], F32, kind="ExternalInput")
    idb_in = nc.dram_tensor("identb", [128, 128], BF16, kind="ExternalInput")
    idx_in = nc.dram_tensor("idx", [128, toti16], I16, kind="ExternalInput")
    padel_in = nc.dram_tensor("padel", [128, 1], F32, kind="ExternalInput")
    out_d = nc.dram_tensor("out", [NPAD, HF], F32, kind="ExternalOutput")

    with tile.TileContext(nc) as tc:
        with (
            tc.tile_pool(name="const", bufs=1) as constp,
            tc.tile_pool(name="dram", bufs=1, space="DRAM") as dramp,
        ):
            tbl_shard = dramp.tile([NPAD, ROWT], U16)
            tbl_halves = [dramp.tile([HALF, ROWT], U16, addr_space="Shared",
                                     name=f"tblh{k}", tag=f"tblh{k}")
                          for k in range(2)]
            dum_in = dramp.tile([8, 16], I16)
            dum_out = dramp.tile([64, 16], I16, addr_space="Shared")

            # tiny warm-up collective: absorbs the one-time cross-core
            # barrier cost while the projection is still running
            nc.sync.dma_start(out=dum_in[:], in_=idx_in[0:8, 0:16])
            nc.gpsimd.collective_compute(
                "AllGather", mybir.AluOpType.bypass,
                replica_groups=[list(range(N_CORES))],
                ins=[dum_in[:].opt()], outs=[dum_out[:].opt()])

            identb = constp.tile([128, 128], BF16)
            nc.sync.dma_start(out=identb[:], in_=idb_in[:, :])
            bias_sb = constp.tile([128, HF], F32)
            nc.sync.dma_start(out=bias_sb[:], in_=bias_in[:, :])
            idx_sb = constp.tile([128, toti16], I16)
            nc.sync.dma_start(out=idx_sb[:], in_=idx_in[:, :])
            alb = constp.tile([128, HF], F32)
            nc.sync.dma_start(out=alb[:], in_=alb_in[:, :])
            arb = constp.tile([128, HF], F32)
            nc.sync.dma_start(out=arb[:], in_=arb_in[:, :])
            padel = constp.tile([128, 1], F32)
            nc.sync.dma_start(out=padel[:], in_=padel_in[:, :])
            c_eps = constp.tile([128, 1], F32)
            nc.vector.memset(c_eps[:], 1e-30)

            # ---- W_aug: [W | Wl | Wr] per k-half (fh layout) ----
            waug = []
            with tc.tile_pool(name="wtmp", bufs=2) as wtmp:
                for kh in range(2):
                    wa = constp.tile([128, 272], BF16, name=f"waug{kh}",
                                     tag=f"waug{kh}")
                    wf = wtmp.tile([128, HF], F32, tag="wf")
                    nc.sync.dma_start(
                        out=wa[:, 0:HF],
                        in_=w_in[kh * 128:(kh + 1) * 128, :])
                    nc.vector.tensor_copy(out=wf[:], in_=wa[:, 0:HF])
                    tmp = wtmp.tile([128, HF], F32, tag="wt")
                    wlr = wtmp.tile([128, 2 * NUM_HEADS], F32, tag="wlr")
                    nc.vector.tensor_mul(out=tmp[:], in0=wf[:], in1=alb[:])
                    nc.vector.tensor_reduce(
                        out=wlr[:, 0:NUM_HEADS],
                        in_=tmp[:].rearrange("p (f h) -> p h f", h=NUM_HEADS),
                        axis=mybir.AxisListType.X, op=mybir.AluOpType.add)
                    tmp2 = wtmp.tile([128, HF], F32, tag="wt2")
                    nc.vector.tensor_mul(out=tmp2[:], in0=wf[:], in1=arb[:])
                    nc.vector.tensor_reduce(
                        out=wlr[:, NUM_HEADS:],
                        in_=tmp2[:].rearrange("p (f h) -> p h f", h=NUM_HEADS),
                        axis=mybir.AxisListType.X, op=mybir.AluOpType.add)
                    nc.vector.tensor_copy(out=wa[:, 256:272], in_=wlr[:])
                    waug.append(wa)

            # er stays resident in SBUF (never round-trips through DRAM)
            er_all = constp.tile([128, NWIN, NUM_HEADS], F32)

            # ---- projection + per-half AllGather ----
            with (
                tc.tile_pool(name="ft", bufs=1) as ftp,
                tc.tile_pool(name="hps", bufs=3, space="PSUM") as hpsp,
                tc.tile_pool(name="hsb", bufs=6) as hsbp,
            ):
                ft_all = []
                for kh in range(2):
                    fta = ftp.tile([128, NPAD], BF16, name=f"fta{kh}",
                                   tag=f"fta{kh}")
                    nc.sync.dma_start(
                        out=fta[:], in_=featT_in[kh * 128:(kh + 1) * 128, :])
                    ft_all.append(fta)
                for t in range(NWIN):
                    hp = hpsp.tile([128, 272], F32, tag="hp")
                    for kh in range(2):
                        nc.tensor.matmul(
                            hp[:],
                            lhsT=ft_all[kh][:, t * 128:(t + 1) * 128],
                            rhs=waug[kh][:, 0:272],
                            start=(kh == 0), stop=(kh == 1))
                    hs = hsbp.tile([128, ROWT], U16, tag="hs")
                    # el (f32, lanes 0-15); pad ranks get PAD_EL
                    if t in PADW:
                        nc.vector.tensor_add(
                            out=hs[:, 0:16].bitcast(F32),
                            in0=hp[:, 256:264],
                            in1=padel[:].to_broadcast([128, NUM_HEADS]))
                    else:
                        nc.vector.tensor_copy(out=hs[:, 0:16].bitcast(F32),
                                              in_=hp[:, 256:264])
                    # h (bf16, lanes 16-271)
                    nc.scalar.copy(out=hs[:, 16:272].bitcast(BF16),
                                   in_=hp[:, 0:256])
                    nc.vector.tensor_copy(out=er_all[:, t, :],
                                          in_=hp[:, 264:272])
                    nc.sync.dma_start(
                        out=tbl_shard[t * 128:(t + 1) * 128, 0:ROWC],
                        in_=hs[:, 0:ROWC])
                    if t % WPC == WPC - 1:
                        k = t // WPC
                        nc.gpsimd.collective_compute(
                            "AllGather", mybir.AluOpType.bypass,
                            replica_groups=[list(range(N_CORES))],
                            ins=[tbl_shard[k * RPC:(k + 1) * RPC, :].opt()],
                            outs=[tbl_halves[k][:].opt()],
                        )

            # ---- aggregation ----
            _agg(nc, tc, constp, sched, idx_sb, er_all, tbl_halves, identb,
                 bias_sb, out_d, queues, c_eps)
    nc.compile()
    return nc


def _agg(nc, tc, constp, sched, idx_sb, er_all, tbl_halves, identb,
         bias_sb, out_d, queues, c_eps):
    halves = [tbl_halves[0][:], tbl_halves[1][:]]

    # all half-A work first (overlaps half-B's AllGather); per-window A
    # sums park in SBUF, half-B pass folds them in and normalizes
    by_wh = [[[] for _ in range(NWIN)] for _ in range(2)]
    for gi, (w, h, cols, off, j0) in enumerate(sched):
        by_wh[h][w].append((gi, cols, off))

    partA = constp.tile([128, NWIN, 264], F32)
    emit = 0                                # gather emission counter

    with (
        tc.tile_pool(name="g", bufs=9) as gp,
        tc.tile_pool(name="m", bufs=8) as mp,
        tc.tile_pool(name="agg", bufs=8, space="PSUM") as aggp,
        tc.tile_pool(name="sm", bufs=3) as smp,
        tc.tile_pool(name="ob", bufs=3) as obp,
    ):
        for h in range(2):
            for w in range(NWIN):
                ncw = sum(cols for (_, cols, _) in by_wh[h][w])
                ps = aggp.tile([128, 264], F32, tag="ps")
                cg = 0
                for (gi, cols, off) in by_wh[h][w]:
                    gt = gp.tile([128, MAXC, ROWT], U16, tag="g")
                    n = 128 * cols
                    nc.gpsimd.dma_gather(
                        gt[:, 0:cols, :], halves[h],
                        idx_sb[:, off:off + cols * 8],
                        n, n, ROWT,
                        queue_num=emit % NSQ, single_packet=False)
                    emit += 1
                    mt = mp.tile([128, MAXC, 264], BF16, tag="m")
                    et = mp.tile([128, MAXC * NUM_HEADS], F32, tag="et")
                    # e = el + er
                    erb = er_all[:, w, :][:, None, :].to_broadcast(
                        [128, cols, NUM_HEADS])
                    etv = et[:, 0:cols * NUM_HEADS].rearrange(
                        "p (c h) -> p c h", h=NUM_HEADS)
                    nc.vector.tensor_add(
                        out=etv, in0=gt[:, 0:cols, 0:16].bitcast(F32),
                        in1=erb)
                    # leaky: x = max(slope*x, x) fused on DVE; exp on the
                    # scalar engine; pexp lands in the rhs tile lanes 256-263
                    efl = et[:, 0:cols * NUM_HEADS]
                    nc.vector.scalar_tensor_tensor(
                        out=efl, in0=efl, scalar=NEG_SLOPE, in1=efl,
                        op0=mybir.AluOpType.mult, op1=mybir.AluOpType.max)
                    pexp = mt[:, 0:cols, 256:264]
                    nc.scalar.activation(
                        out=pexp, in_=etv,
                        func=mybir.ActivationFunctionType.Exp)
                    # msg = h * pexp (fh layout: broadcast over f, packed h)
                    outv = mt[:, 0:cols, 0:HF].rearrange(
                        "p c (f h) -> p c f h", h=NUM_HEADS)
                    in0v = gt[:, 0:cols, 16:272].bitcast(BF16).rearrange(
                        "p c (f h) -> p c f h", h=NUM_HEADS)
                    in1v = pexp[:, :, None, :] \
                        .to_broadcast([128, cols, OUT_FEATS, NUM_HEADS])
                    nc.vector.tensor_mul(out=outv, in0=in0v, in1=in1v)
                    for ci in range(cols):
                        nc.tensor.matmul(
                            ps[:], lhsT=identb[:], rhs=mt[:, ci, :],
                            start=(cg == 0), stop=(cg == ncw - 1))
                        cg += 1
                if h == 0:
                    nc.scalar.copy(out=partA[:, w, :], in_=ps[:])
                    continue
                # fold in the half-A partial and normalize
                fs = obp.tile([128, 264], F32, tag="fs")
                nc.vector.tensor_add(out=fs[:], in0=ps[:],
                                     in1=partA[:, w, :])
                sp = smp.tile([128, NUM_HEADS], F32, tag="sp")
                nc.vector.tensor_add(
                    out=sp[:], in0=fs[:, 256:264],
                    in1=c_eps[:].to_broadcast([128, NUM_HEADS]))
                rp = smp.tile([128, NUM_HEADS], F32, tag="rp")
                nc.vector.reciprocal(out=rp[:], in_=sp[:])
                ob = obp.tile([128, HF], F32, tag="ob")
                nc.vector.tensor_mul(
                    out=ob[:].rearrange("p (f h) -> p f h", h=NUM_HEADS),
                    in0=fs[:, 0:HF].rearrange("p (f h) -> p f h",
                                              h=NUM_HEADS),
                    in1=rp[:][:, None, :].to_broadcast(
                        [128, OUT_FEATS, NUM_HEADS]))
                nc.vector.tensor_add(out=ob[:], in0=ob[:], in1=bias_sb[:])
                nc.sync.dma_start(
                    out=out_d[w * 128:(w + 1) * 128, :], in_=ob[:])


def _fh_perm():
    """Column permutation hf -> fh: new lane f*H+h takes old lane h*F+f."""
    f, h = np.meshgrid(np.arange(OUT_FEATS), np.arange(NUM_HEADS),
                       indexing="ij")
    return (h * OUT_FEATS + f).ravel()     # [256] old-lane index per new lane


def kernel(feat, W, attn_l, attn_r, bias, src, dst):
    import ml_dtypes
    feat = np.asarray(feat, dtype=np.float32)
    W = np.asarray(W, dtype=np.float32)
    attn_l = np.asarray(attn_l, dtype=np.float32)
    attn_r = np.asarray(attn_r, dtype=np.float32)
    bias = np.asarray(bias, dtype=np.float32)
    src = np.asarray(src).astype(np.int64)
    dst = np.asarray(dst).astype(np.int64)

    perms, sched, toti16, idx_imgs, ncE, ncO, queues = _prep(src, dst)
    nc = _build(sched, toti16, ncE, ncO, queues)

    fh = _fh_perm()
    W_fh = W[:, fh].astype(ml_dtypes.bfloat16)
    al_fh = attn_l.reshape(HF)[fh]
    ar_fh = attn_r.reshape(HF)[fh]
    bias_fh = bias.reshape(HF)[fh]
    alb = np.tile(al_fh.reshape(1, HF), (128, 1)).astype(np.float32)
    arb = np.tile(ar_fh.reshape(1, HF), (128, 1)).astype(np.float32)
    biasb = np.tile(bias_fh.reshape(1, HF), (128, 1)).astype(np.float32)
    identb = np.eye(128).astype(ml_dtypes.bfloat16)
    padel = np.zeros((128, 1), np.float32)
    padel[PAD_P:] = PAD_EL

    in_maps = []
    for c in range(N_CORES):
        fc = np.zeros((NPAD, IN_FEATS), np.float32)
        real = perms[c] >= 0
        fc[real] = feat[perms[c][real]]
        fcT = np.ascontiguousarray(fc.T).astype(ml_dtypes.bfloat16)
        in_maps.append({
            "featT": fcT, "w": W_fh, "alb": alb, "arb": arb,
            "biasb": biasb, "identb": identb, "idx": idx_imgs[c],
            "padel": padel,
        })
    res = run_bass_kernel_spmd(nc, in_maps, core_ids=list(range(N_CORES)),
                               trace=False)
    out = np.empty((N_NODES, HF), np.float32)
    for c in range(N_CORES):
        r = res.results[c]["out"]
        real = perms[c] >= 0
        out[perms[c][real]] = r[real]
    # undo fh lane layout -> [N, H, F]
    return np.ascontiguousarray(
        out.reshape(N_NODES, OUT_FEATS, NUM_HEADS).transpose(0, 2, 1))


if __name__ == "__main__":
    rng = np.random.default_rng(0)
    feat = rng.standard_normal((N_NODES, IN_FEATS), np.float32)
    W = (rng.standard_normal((IN_FEATS, HF), np.float32) * 0.05)
    al = rng.standard_normal((NUM_HEADS, OUT_FEATS), np.float32) * 0.1
    ar = rng.standard_normal((NUM_HEADS, OUT_FEATS), np.float32) * 0.1
    b = np.zeros((NUM_HEADS, OUT_FEATS), np.float32)
    src = rng.integers(0, N_NODES, N_EDGES)
    dst = rng.integers(0, N_NODES, N_EDGES)
    out = kernel(feat=feat, W=W, attn_l=al, attn_r=ar, bias=b,
                 src=src, dst=dst)
    print("out", out.shape, out.dtype, np.abs(out).mean())


# revision 39
# speedup vs baseline: 1.0597x; 1.0597x over previous
"""GAT layer on 8 Trainium2 NeuronCores (Bass/Tile).

Strategy (edge partition by dst, 1D graph parallelism), v2:
- Each core owns 6250 dst nodes; nodes re-ordered by in-degree so each
  128-node PSUM window has near-uniform degree; edge slot (p, c) holds the
  c-th incoming edge of window-node p, so the segment scatter-sum is a
  matmul-accumulate with an identity stationary operand and er[dst]
  broadcasts per-partition.
- Projection h = featT.T @ [W | Wl | Wr] is node-parallel in bf16 with the
  feat tensor pre-transposed host-side (no on-chip transposes).  h is
  stored feature-major (lane = f*8 + head) so the per-edge softmax scale
  broadcast has a packed inner dim (DVE 2x mode).
- Per-node table row (768 B pitch): [el f32 32B | h bf16 512B | pad].
  The table is AllGathered in 7 chunks overlapped with the projection.
- Table halves for int16 gather indices split by rank parity (even/odd
  rows via stride-1536B views); within each window the 128 nodes are
  split 64/64 into parity classes balancing every dst node's in-edges
  across halves (keeps per-window grid columns ~deg/2 per half).
- leaky-relu and exp run on the scalar engine; softmax is unnormalized
  (logits are O(1)) and normalization happens per node after aggregation.
"""
import math
import numpy as np
import sys

sys.path.insert(0, "/opt/trn_rl_repo")

from concourse import bass, mybir, bacc, tile
from concourse.bass_utils import run_bass_kernel_spmd

N_NODES = 50000
N_EDGES = 800000
IN_FEATS = 256
NUM_HEADS = 8
OUT_FEATS = 32
HF = NUM_HEADS * OUT_FEATS          # 256
NEG_SLOPE = 0.2
N_CORES = 8
NPAD = 6400                         # 50 * 128 ranks per core
NWIN = NPAD // 128                  # 50
NCHUNK = 2                          # AllGather chunks == table halves
WPC = NWIN // NCHUNK                # 25 windows per chunk
RPC = WPC * 128                     # 3200 rows per (core, chunk)
ROWT = 384                          # table row u16 lanes (768 B):
                                    #   lane 0-15:    el f32 (8 heads)
                                    #   lane 16-271:  h bf16 (fh layout)
                                    #   lane 272-383: pad
TROWS = N_CORES * NPAD              # 51200
HALF = TROWS // 2                   # 25600 rows per half (chunks 0-4 / 5-9)
HRANK = NPAD // 2                   # 3200 ranks per half per core
HREAL = N_NODES // (2 * N_CORES)    # 3125 real nodes per (core, half)
PAD_P = HREAL - (HRANK // 128 - 1) * 128   # 53: pad partition threshold
PADW = (NWIN // 2 - 1, NWIN - 1)    # windows 24 and 49 hold the pad ranks
PAD_EL = -1e30                      # pad rows' el -> exp() == 0
MAXC = 16                           # chunk columns per dma_gather
NSQ = 4                             # SWDGE queues
ROWC = 272                          # used lanes per row (el + h)
F32 = mybir.dt.float32
BF16 = mybir.dt.bfloat16
U16 = mybir.dt.uint16
I16 = mybir.dt.int16


def _row_of(core, rank):
    """Table row index for (core, rank-within-core) under chunked AG."""
    k = rank // RPC
    return k * (N_CORES * RPC) + core * RPC + (rank % RPC)


def _assign(src, dst):
    """Assign each node a (core, half).  Nodes are processed in degree-desc
    groups of 16; each group fills the 16 (core, half) slots, the greedy
    choosing each node's half to balance every dst node's in-edge count
    across table halves, followed by swap-refinement sweeps.  Every
    (core, half) holds exactly HREAL real nodes (stratified by degree)."""
    deg = np.bincount(dst, minlength=N_NODES)
    order_by_src = np.argsort(src, kind="stable")
    ssd = dst[order_by_src]
    starts = np.searchsorted(src[order_by_src], np.arange(N_NODES + 1))

    gorder = np.argsort(-deg, kind="stable")
    imb = np.zeros(N_NODES, np.int32)       # degA - degB per dst node
    core_of = np.empty(N_NODES, np.int64)
    half_of = np.empty(N_NODES, np.int64)
    for g in range(HREAL):
        grp = gorder[g * 16:(g + 1) * 16]
        capA = capB = 8
        for v in grp:
            nb = ssd[starts[v]:starts[v + 1]]
            if capA == 0:
                toA = False
            elif capB == 0:
                toA = True
            elif len(nb) == 0:
                toA = capA >= capB
            else:
                cur = imb[nb]
                toA = np.abs(cur + 1).sum() <= np.abs(cur - 1).sum()
            if toA:
                capA -= 1
                core_of[v] = 7 - capA
                half_of[v] = 0
                if len(nb):
                    imb[nb] += 1
            else:
                capB -= 1
                core_of[v] = 7 - capB
                half_of[v] = 1
                if len(nb):
                    imb[nb] -= 1
    # swap-refinement: within each group, swap the best (A, B) pair's
    # (core, half) labels while it reduces total imbalance
    for sweep in range(6):
        nswap = 0
        for g in range(HREAL):
            grp = gorder[g * 16:(g + 1) * 16]
            ha = half_of[grp]
            A = grp[ha == 0]
            B = grp[ha == 1]
            gA = np.empty(len(A))
            gB = np.empty(len(B))
            for i, u in enumerate(A):
                cur = imb[ssd[starts[u]:starts[u + 1]]]
                gA[i] = (np.abs(cur - 2) - np.abs(cur)).sum()
            for i, v in enumerate(B):
                cur = imb[ssd[starts[v]:starts[v + 1]]]
                gB[i] = (np.abs(cur + 2) - np.abs(cur)).sum()
            iu = int(gA.argmin())
            iv = int(gB.argmin())
            if gA[iu] + gB[iv] < 0:
                u, v = A[iu], B[iv]
                imb[ssd[starts[u]:starts[u + 1]]] -= 2
                imb[ssd[starts[v]:starts[v + 1]]] += 2
                half_of[u], half_of[v] = 1, 0
                core_of[u], core_of[v] = core_of[v], core_of[u]
                nswap += 1
        if nswap == 0:
            break
    return core_of, half_of


def _grid_for(p_arr, rel_arr):
    """Edges (partition p, rel table idx) -> grid [128, ncx] int32,
    pad slots = -1.  Each partition's edges sorted ascending by rel."""
    if len(p_arr) == 0:
        return np.full((128, 1), -1, np.int32)
    o = np.lexsort((rel_arr, p_arr))
    p_arr, rel_arr = p_arr[o], rel_arr[o]
    counts = np.bincount(p_arr, minlength=128)
    ncx = int(counts.max())
    grid = np.full((128, ncx), -1, np.int32)
    starts = np.zeros(128, np.int64)
    starts[1:] = np.cumsum(counts)[:-1]
    col = np.arange(len(p_arr)) - starts[p_arr]
    grid[p_arr, col] = rel_arr
    return grid


def _pack_gather(grid_cols):
    """grid_cols [128, cols] -> wrapped idx image [128, n/16] int16."""
    cols = grid_cols.shape[1]
    n = 128 * cols
    flat = np.empty(n, np.int32)
    P = np.arange(128)[:, None]
    CI = np.arange(cols)[None, :]
    pos = (P % 16) * (n // 16) + (P // 16) + 8 * CI
    flat[pos.ravel()] = grid_cols.ravel()
    return np.tile(flat.reshape(16, n // 16), (8, 1)).astype(np.int16)


def _prep(src, dst):
    """Host-side index preprocessing.  Returns per-core node perms + the
    shared gather schedule + per-core idx images."""
    core_of, half_of = _assign(src, dst)

    # rank within each (core, half) by (max(degA,degB) desc, min desc): the
    # grid column count per (window, half) is the window max of each
    # half-degree, so grouping by the worst-half degree minimizes both
    srcA = half_of[src] == 0
    degA = np.bincount(dst[srcA], minlength=N_NODES)
    degB = np.bincount(dst[~srcA], minlength=N_NODES)
    kmax = np.maximum(degA, degB)
    kmin = np.minimum(degA, degB)
    rank_of = np.empty(N_NODES, np.int64)
    for c in range(N_CORES):
        for h in range(2):
            ids = np.nonzero((core_of == c) & (half_of == h))[0]
            order = np.lexsort((-kmin[ids], -kmax[ids]))
            rank_of[ids[order]] = h * HRANK + np.arange(len(ids))

    perms = []                              # rank -> global node id
    for c in range(N_CORES):
        perm = np.full(NPAD, -1, np.int64)
        ids = np.nonzero(core_of == c)[0]
        perm[rank_of[ids]] = ids
        perms.append(perm)

    row_all = (rank_of // RPC) * (N_CORES * RPC) \
        + core_of * RPC + (rank_of % RPC)

    sg_half = (rank_of[src] >= HRANK).astype(np.int64)
    sg_rel = row_all[src] - sg_half * HALF  # rel row within half
    dcore = core_of[dst]
    dr_all = rank_of[dst]                   # local rank of dst

    # per (core, window, half) grids
    grids = [[[None, None] for _ in range(NWIN)] for _ in range(N_CORES)]
    ncx = np.zeros((N_CORES, NWIN, 2), np.int64)
    for c in range(N_CORES):
        sel = np.nonzero(dcore == c)[0]
        rel = sg_rel[sel]
        half = sg_half[sel]
        dr = dr_all[sel]
        w_arr = dr // 128
        p_arr = dr % 128
        key = w_arr * 2 + half
        order = np.argsort(key, kind="stable")
        ksort = key[order]
        bounds = np.searchsorted(ksort, np.arange(NWIN * 2 + 1))
        for w in range(NWIN):
            for h in range(2):
                lo, hi = bounds[w * 2 + h], bounds[w * 2 + h + 1]
                idxs = order[lo:hi]
                g = _grid_for(p_arr[idxs].astype(np.int64),
                              rel[idxs].astype(np.int64))
                grids[c][w][h] = g
                ncx[c, w, h] = g.shape[1]

    # shared schedule: per (window, half) chunk count = max over cores
    ncE = np.maximum(ncx[:, :, 0].max(axis=0), 1)
    ncO = np.maximum(ncx[:, :, 1].max(axis=0), 1)
    sched = []                              # (w, half, cols, off16)
    off16 = 0
    for w in range(NWIN):
        for h, nc_w in ((0, int(ncE[w])), (1, int(ncO[w]))):
            for j0 in range(0, nc_w, MAXC):
                cols = min(MAXC, nc_w - j0)
                sched.append((w, h, cols, off16, j0))
                off16 += cols * 8           # n/16 = 128*cols/16
    toti16 = off16
    queues = [i % NSQ for i in range(len(sched))]

    # pad slots rotate over all pad rows of the matching half (ranks
    # HREAL..HRANK / HRANK+HREAL..NPAD) to dodge same-bank serialization.
    padpools = []
    for h in range(2):
        pool = []
        for c in range(N_CORES):
            for r in range(h * HRANK + HREAL, (h + 1) * HRANK):
                pool.append(_row_of(c, r) - h * HALF)
        padpools.append(np.array(pool, np.int32))

    idx_imgs = []
    for c in range(N_CORES):
        img = np.empty((128, toti16), np.int16)
        phase = 0
        for (w, h, cols, off, j0) in sched:
            g = grids[c][w][h]
            gc = np.full((128, cols), -1, np.int32)
            avail = max(0, min(cols, g.shape[1] - j0))
            if avail > 0:
                gc[:, :avail] = g[:, j0:j0 + avail]
            mask = gc < 0
            npads = int(mask.sum())
            if npads:
                pool = padpools[h]
                gc[mask] = pool[(np.arange(npads) + phase) % len(pool)]
                phase += npads
            img[:, off:off + cols * 8] = _pack_gather(gc)
        idx_imgs.append(img)
    return perms, sched, toti16, idx_imgs, ncE, ncO, queues


def _build(sched, toti16, ncE, ncO, queues):
    nc = bacc.Bacc("TRN2", target_bir_lowering=False, debug=False,
                   num_devices=N_CORES, num_swdge_queues=NSQ)
    featT_in = nc.dram_tensor("featT", [IN_FEATS, NPAD], BF16,
                              kind="ExternalInput")
    w_in = nc.dram_tensor("w", [IN_FEATS, HF], BF16, kind="ExternalInput")
    alb_in = nc.dram_tensor("alb", [128, HF], F32, kind="ExternalInput")
    arb_in = nc.dram_tensor("arb", [128, HF], F32, kind="ExternalInput")
    bias_in = nc.dram_tensor("biasb", [128, HF], F32, kind="ExternalInput")
    idb_in = nc.dram_tensor("identb", [128, 128], BF16, kind="ExternalInput")
    idx_in = nc.dram_tensor("idx", [128, toti16], I16, kind="ExternalInput")
    padel_in = nc.dram_tensor("padel", [128, 1], F32, kind="ExternalInput")
    out_d = nc.dram_tensor("out", [NPAD, HF], F32, kind="ExternalOutput")

    with tile.TileContext(nc) as tc:
        with (
            tc.tile_pool(name="const", bufs=1) as constp,
            tc.tile_pool(name="dram", bufs=1, space="DRAM") as dramp,
        ):
            tbl_shard = dramp.tile([NPAD, ROWT], U16)
            tbl_halves = [dramp.tile([HALF, ROWT], U16, addr_space="Shared",
                                     name=f"tblh{k}", tag=f"tblh{k}")
                          for k in range(2)]


            identb = constp.tile([128, 128], BF16)
            nc.sync.dma_start(out=identb[:], in_=idb_in[:, :])
            bias_sb = constp.tile([128, HF], F32)
            nc.sync.dma_start(out=bias_sb[:], in_=bias_in[:, :])
            idx_sb = constp.tile([128, toti16], I16)
            nc.sync.dma_start(out=idx_sb[:], in_=idx_in[:, :])
            alb = constp.tile([128, HF], F32)
            nc.sync.dma_start(out=alb[:], in_=alb_in[:, :])
            arb = constp.tile([128, HF], F32)
            nc.sync.dma_start(out=arb[:], in_=arb_in[:, :])
            padel = constp.tile([128, 1], F32)
            nc.sync.dma_start(out=padel[:], in_=padel_in[:, :])
            c_eps = constp.tile([128, 1], F32)
            nc.vector.memset(c_eps[:], 1e-30)

            # ---- W_aug: [W | Wl | Wr] per k-half (fh layout) ----
            waug = []
            with tc.tile_pool(name="wtmp", bufs=2) as wtmp:
                for kh in range(2):
                    wa = constp.tile([128, 272], BF16, name=f"waug{kh}",
                                     tag=f"waug{kh}")
                    wf = wtmp.tile([128, HF], F32, tag="wf")
                    nc.sync.dma_start(
                        out=wa[:, 0:HF],
                        in_=w_in[kh * 128:(kh + 1) * 128, :])
                    nc.vector.tensor_copy(out=wf[:], in_=wa[:, 0:HF])
                    tmp = wtmp.tile([128, HF], F32, tag="wt")
                    wlr = wtmp.tile([128, 2 * NUM_HEADS], F32, tag="wlr")
                    nc.vector.tensor_mul(out=tmp[:], in0=wf[:], in1=alb[:])
                    nc.vector.tensor_reduce(
                        out=wlr[:, 0:NUM_HEADS],
                        in_=tmp[:].rearrange("p (f h) -> p h f", h=NUM_HEADS),
                        axis=mybir.AxisListType.X, op=mybir.AluOpType.add)
                    tmp2 = wtmp.tile([128, HF], F32, tag="wt2")
                    nc.vector.tensor_mul(out=tmp2[:], in0=wf[:], in1=arb[:])
                    nc.vector.tensor_reduce(
                        out=wlr[:, NUM_HEADS:],
                        in_=tmp2[:].rearrange("p (f h) -> p h f", h=NUM_HEADS),
                        axis=mybir.AxisListType.X, op=mybir.AluOpType.add)
                    nc.vector.tensor_copy(out=wa[:, 256:272], in_=wlr[:])
                    waug.append(wa)

            # er stays resident in SBUF (never round-trips through DRAM)
            er_all = constp.tile([128, NWIN, NUM_HEADS], F32)

            # ---- projection + per-half AllGather ----
            with (
                tc.tile_pool(name="ft", bufs=1) as ftp,
                tc.tile_pool(name="hps", bufs=4, space="PSUM") as hpsp,
                tc.tile_pool(name="hsb", bufs=8) as hsbp,
            ):
                ft_all = []
                for kh in range(2):
                    fta = ftp.tile([128, NPAD], BF16, name=f"fta{kh}",
                                   tag=f"fta{kh}")
                    nc.sync.dma_start(
                        out=fta[:], in_=featT_in[kh * 128:(kh + 1) * 128, :])
                    ft_all.append(fta)
                for t in range(NWIN):
                    hp = hpsp.tile([128, 272], F32, tag="hp")
                    for kh in range(2):
                        nc.tensor.matmul(
                            hp[:],
                            lhsT=ft_all[kh][:, t * 128:(t + 1) * 128],
                            rhs=waug[kh][:, 0:272],
                            start=(kh == 0), stop=(kh == 1))
                    hs = hsbp.tile([128, ROWT], U16, tag="hs")
                    # el (f32, lanes 0-15); pad ranks get PAD_EL
                    if t in PADW:
                        nc.vector.tensor_add(
                            out=hs[:, 0:16].bitcast(F32),
                            in0=hp[:, 256:264],
                            in1=padel[:].to_broadcast([128, NUM_HEADS]))
                    else:
                        nc.vector.tensor_copy(out=hs[:, 0:16].bitcast(F32),
                                              in_=hp[:, 256:264])
                    # h (bf16, lanes 16-271); alternate engines so neither
                    # the scalar nor vector engine serializes the pipeline
                    if t % 2 == 0:
                        nc.scalar.copy(out=hs[:, 16:272].bitcast(BF16),
                                       in_=hp[:, 0:256])
                    else:
                        nc.vector.tensor_copy(
                            out=hs[:, 16:272].bitcast(BF16),
                            in_=hp[:, 0:256])
                    nc.vector.tensor_copy(out=er_all[:, t, :],
                                          in_=hp[:, 264:272])
                    nc.sync.dma_start(
                        out=tbl_shard[t * 128:(t + 1) * 128, 0:ROWC],
                        in_=hs[:, 0:ROWC])
                    if t % WPC == WPC - 1:
                        k = t // WPC
                        nc.gpsimd.collective_compute(
                            "AllGather", mybir.AluOpType.bypass,
                            replica_groups=[list(range(N_CORES))],
                            ins=[tbl_shard[k * RPC:(k + 1) * RPC, :].opt()],
                            outs=[tbl_halves[k][:].opt()],
                        )

            # ---- aggregation ----
            _agg(nc, tc, constp, sched, idx_sb, er_all, tbl_halves, identb,
                 bias_sb, out_d, queues, c_eps)
    nc.compile()
    return nc


def _agg(nc, tc, constp, sched, idx_sb, er_all, tbl_halves, identb,
         bias_sb, out_d, queues, c_eps):
    halves = [tbl_halves[0][:], tbl_halves[1][:]]

    # all half-A work first (overlaps half-B's AllGather); per-window A
    # sums park in SBUF, half-B pass folds them in and normalizes
    by_wh = [[[] for _ in range(NWIN)] for _ in range(2)]
    for gi, (w, h, cols, off, j0) in enumerate(sched):
        by_wh[h][w].append((gi, cols, off))

    partA = constp.tile([128, NWIN, 264], F32)
    emit = 0                                # gather emission counter

    with (
        tc.tile_pool(name="g", bufs=5) as gp,
        tc.tile_pool(name="m", bufs=5) as mp,
        tc.tile_pool(name="agg", bufs=8, space="PSUM") as aggp,
        tc.tile_pool(name="sm", bufs=3) as smp,
        tc.tile_pool(name="ob", bufs=3) as obp,
    ):
        for h in range(2):
            # A pass runs lightest windows first: it overlaps half-B's
            # AllGather, which competes for HBM/DMA bandwidth
            worder = list(reversed(range(NWIN))) if h == 0 else range(NWIN)
            for w in worder:
                ncw = sum(cols for (_, cols, _) in by_wh[h][w])
                ps = aggp.tile([128, 264], F32, tag="ps")
                cg = 0
                for (gi, cols, off) in by_wh[h][w]:
                    gt = gp.tile([128, MAXC, ROWT], U16, tag="g")
                    n = 128 * cols
                    nc.gpsimd.dma_gather(
                        gt[:, 0:cols, :], halves[h],
                        idx_sb[:, off:off + cols * 8],
                        n, n, ROWT,
                        queue_num=emit % NSQ, single_packet=False)
                    emit += 1
                    mt = mp.tile([128, MAXC, 264], BF16, tag="m")
                    et = mp.tile([128, MAXC * NUM_HEADS], F32, tag="et")
                    # e = el + er
                    erb = er_all[:, w, :][:, None, :].to_broadcast(
                        [128, cols, NUM_HEADS])
                    etv = et[:, 0:cols * NUM_HEADS].rearrange(
                        "p (c h) -> p c h", h=NUM_HEADS)
                    nc.vector.tensor_add(
                        out=etv, in0=gt[:, 0:cols, 0:16].bitcast(F32),
                        in1=erb)
                    # leaky: x = max(slope*x, x) fused on DVE; exp on the
                    # scalar engine; pexp lands in the rhs tile lanes 256-263
                    efl = et[:, 0:cols * NUM_HEADS]
                    nc.vector.scalar_tensor_tensor(
                        out=efl, in0=efl, scalar=NEG_SLOPE, in1=efl,
                        op0=mybir.AluOpType.mult, op1=mybir.AluOpType.max)
                    pexp = mt[:, 0:cols, 256:264]
                    nc.scalar.activation(
                        out=pexp, in_=etv,
                        func=mybir.ActivationFunctionType.Exp)
                    # msg = h * pexp (fh layout: broadcast over f, packed h)
                    outv = mt[:, 0:cols, 0:HF].rearrange(
                        "p c (f h) -> p c f h", h=NUM_HEADS)
                    in0v = gt[:, 0:cols, 16:272].bitcast(BF16).rearrange(
                        "p c (f h) -> p c f h", h=NUM_HEADS)
                    in1v = pexp[:, :, None, :] \
                        .to_broadcast([128, cols, OUT_FEATS, NUM_HEADS])
                    nc.vector.tensor_mul(out=outv, in0=in0v, in1=in1v)
                    for ci in range(cols):
                        nc.tensor.matmul(
                            ps[:], lhsT=identb[:], rhs=mt[:, ci, :],
                            start=(cg == 0), stop=(cg == ncw - 1))
                        cg += 1
                if h == 0:
                    nc.scalar.copy(out=partA[:, w, :], in_=ps[:])
                    continue
                # fold in the half-A partial and normalize
                fs = obp.tile([128, 264], F32, tag="fs")
                nc.vector.tensor_add(out=fs[:], in0=ps[:],
                                     in1=partA[:, w, :])
                sp = smp.tile([128, NUM_HEADS], F32, tag="sp")
                nc.vector.tensor_add(
                    out=sp[:], in0=fs[:, 256:264],
                    in1=c_eps[:].to_broadcast([128, NUM_HEADS]))
                rp = smp.tile([128, NUM_HEADS], F32, tag="rp")
                nc.vector.reciprocal(out=rp[:], in_=sp[:])
                ob = obp.tile([128, HF], F32, tag="ob")
                nc.vector.tensor_mul(
                    out=ob[:].rearrange("p (f h) -> p f h", h=NUM_HEADS),
                    in0=fs[:, 0:HF].rearrange("p (f h) -> p f h",
                                              h=NUM_HEADS),
                    in1=rp[:][:, None, :].to_broadcast(
                        [128, OUT_FEATS, NUM_HEADS]))
                nc.vector.tensor_add(out=ob[:], in0=ob[:], in1=bias_sb[:])
                nc.sync.dma_start(
                    out=out_d[w * 128:(w + 1) * 128, :], in_=ob[:])


def _fh_perm():
    """Column permutation hf -> fh: new lane f*H+h takes old lane h*F+f."""
    f, h = np.meshgrid(np.arange(OUT_FEATS), np.arange(NUM_HEADS),
                       indexing="ij")
    return (h * OUT_FEATS + f).ravel()     # [256] old-lane index per new lane


def kernel(feat, W, attn_l, attn_r, bias, src, dst):
    import ml_dtypes
    feat = np.asarray(feat, dtype=np.float32)
    W = np.asarray(W, dtype=np.float32)
    attn_l = np.asarray(attn_l, dtype=np.float32)
    attn_r = np.asarray(attn_r, dtype=np.float32)
    bias = np.asarray(bias, dtype=np.float32)
    src = np.asarray(src).astype(np.int64)
    dst = np.asarray(dst).astype(np.int64)

    perms, sched, toti16, idx_imgs, ncE, ncO, queues = _prep(src, dst)
    nc = _build(sched, toti16, ncE, ncO, queues)

    fh = _fh_perm()
    W_fh = W[:, fh].astype(ml_dtypes.bfloat16)
    al_fh = attn_l.reshape(HF)[fh]
    ar_fh = attn_r.reshape(HF)[fh]
    bias_fh = bias.reshape(HF)[fh]
    alb = np.tile(al_fh.reshape(1, HF), (128, 1)).astype(np.float32)
    arb = np.tile(ar_fh.reshape(1, HF), (128, 1)).astype(np.float32)
    biasb = np.tile(bias_fh.reshape(1, HF), (128, 1)).astype(np.float32)
    identb = np.eye(128).astype(ml_dtypes.bfloat16)
    padel = np.zeros((128, 1), np.float32)
    padel[PAD_P:] = PAD_EL

    in_maps = []
    for c in range(N_CORES):
        fc = np.zeros((NPAD, IN_FEATS), np.float32)
        real = perms[c] >= 0
        fc[real] = feat[perms[c][real]]
        fcT = np.ascontiguousarray(fc.T).astype(ml_dtypes.bfloat16)
        in_maps.append({
            "featT": fcT, "w": W_fh, "alb": alb, "arb": arb,
            "biasb": biasb, "identb": identb, "idx": idx_imgs[c],
            "padel": padel,
        })
    res = run_bass_kernel_spmd(nc, in_maps, core_ids=list(range(N_CORES)),
                               trace=False)
    out = np.empty((N_NODES, HF), np.float32)
    for c in range(N_CORES):
        r = res.results[c]["out"]
        real = perms[c] >= 0
        out[perms[c][real]] = r[real]
    # undo fh lane layout -> [N, H, F]
    return np.ascontiguousarray(
        out.reshape(N_NODES, OUT_FEATS, NUM_HEADS).transpose(0, 2, 1))


if __name__ == "__main__":
    rng = np.random.default_rng(0)
    feat = rng.standard_normal((N_NODES, IN_FEATS), np.float32)
    W = (rng.standard_normal((IN_FEATS, HF), np.float32) * 0.05)
    al = rng.standard_normal((NUM_HEADS, OUT_FEATS), np.float32) * 0.1
    ar = rng.standard_normal((NUM_HEADS, OUT_FEATS), np.float32) * 0.1
    b = np.zeros((NUM_HEADS, OUT_FEATS), np.float32)
    src = rng.integers(0, N_NODES, N_EDGES)
    dst = rng.integers(0, N_NODES, N_EDGES)
    out = kernel(feat=feat, W=W, attn_l=al, attn_r=ar, bias=b,
                 src=src, dst=dst)
    print("out", out.shape, out.dtype, np.abs(out).mean())
